# revision 3
# baseline (speedup 1.0000x reference)
"""Trainium2 Bass kernel for nn_MultiHeadAttention_66486093742402.

Reference computation (note source bug faithfully reproduced: v is projected
with WK; WV is unused):
    qp = q @ WQ.T ; kp = k @ WK.T ; vp = v @ WK.T          (per batch b)
    per head h: scores = (qh @ kh.T) / 8 ; att = softmax(scores)
                sdpa = att @ vh
    out[b,s,h*64+d] = sdpa[h,s,d] ;  returns (out, att)

Sharding: 8 cores = 4 batches x 2 head-groups (8 heads each).
Per core inputs (host-prepared, transposed):
    qT,kT,vT [1024,2048] = q[b].T ...,  wqT,wkT [1024,512] = W[g*512:(g+1)*512].T
Per core outputs: att [8,2048,2048], out [2048,512].

On-chip plan (per core):
  projections:  qpT/kpT [512e, 2048s] (e on partitions), vp [2048s, 512e]
  per head h (e-tile m=h//2, base partition bp=(h%2)*64):
    Phase A (att output): per q-tile: scores[128q,2048k] = qpT_blk.T @ kpT,
       exp via ScalarE (scale=0.125, accum_out -> denom), recip, normalize
       (VectorE), DMA att row-block (contiguous 1MB).
    Phase B (sdpa): per k-tile: scoresT[128k,2048q] = kpT_blk.T @ qpT, exp ->
       attuT, accumulate outT[64dh,2048q] = vh.T @ attuT over k-tiles in PSUM
       ([64,1024] x2 halves), then PE-transpose 128x64 blocks, normalize by
       recip into out staging, single 4MB DMA at end.
All matmuls run as float32r (full-rate fp32; set KERNEL_MM_DT=float32 for the
4x-slower exact-fp32 fallback).
"""

import os
import sys
from contextlib import ExitStack

import numpy as np

sys.path.insert(0, "/opt/trn_rl_repo")

P = 128          # partitions
S = 2048         # sequence
D = 1024         # model dim
E = 512          # per-core output dim (8 heads x 64)
HD = 64          # head depth
NH = 8           # heads per core
ST = S // P      # 16 seq tiles
DT = D // P      # 8 contraction tiles
ET = E // P      # 4 e-tiles
N_CORES = 8

_CACHE = {}


def _build_nc(mm_dt: str):
    import concourse.tile as tile
    from concourse import bacc, mybir
    from concourse.masks import make_identity

    f32 = mybir.dt.float32
    mmdt = getattr(mybir.dt, mm_dt)
    EXP = mybir.ActivationFunctionType.Exp

    def mm(ap):
        return ap

    nc = bacc.Bacc("TRN2", target_bir_lowering=False, debug=False,
                   enable_asserts=False)

    qT = nc.dram_tensor("qT", [D, S], mmdt, kind="ExternalInput").ap()
    kT = nc.dram_tensor("kT", [D, S], mmdt, kind="ExternalInput").ap()
    vT = nc.dram_tensor("vT", [D, S], mmdt, kind="ExternalInput").ap()
    wqT = nc.dram_tensor("wqT", [D, E], mmdt, kind="ExternalInput").ap()
    wkT = nc.dram_tensor("wkT", [D, E], mmdt, kind="ExternalInput").ap()
    att = nc.dram_tensor("att", [NH, S, S], f32, kind="ExternalOutput").ap()
    out = nc.dram_tensor("out", [S, E], f32, kind="ExternalOutput").ap()

    with tile.TileContext(nc) as tc, ExitStack() as ctx:
        persist = ctx.enter_context(tc.tile_pool(name="persist", bufs=1))
        qpT = [persist.tile([P, S], mmdt, tag=f"qpT{m}", name=f"qpT{m}")
               for m in range(ET)]
        kpT = [persist.tile([P, S], mmdt, tag=f"kpT{m}", name=f"kpT{m}")
               for m in range(ET)]
        vp = [persist.tile([P, E], mmdt, tag=f"vp{t}", name=f"vp{t}")
              for t in range(ST)]
        outs = persist.tile([P, ST, E], f32, tag="outs", name="outs")
        recip = persist.tile([P, NH, ST], f32, tag="recip", name="recip")
        ident = persist.tile([P, P], f32, tag="ident", name="ident")
        make_identity(nc, ident)

        # ------------------------------------------------ projections
        with ExitStack() as pctx:
            wpool = pctx.enter_context(tc.tile_pool(name="wpool", bufs=1))
            wq = wpool.tile([P, DT, E], mmdt, tag="wq", name="wq")
            wk = wpool.tile([P, DT, E], mmdt, tag="wk", name="wk")
            nc.sync.dma_start(wq, wqT.rearrange("(t p) e -> p t e", p=P))
            nc.sync.dma_start(wk, wkT.rearrange("(t p) e -> p t e", p=P))

            inp = pctx.enter_context(tc.tile_pool(name="inp", bufs=3))
            pps = pctx.enter_context(
                tc.tile_pool(name="pps", bufs=2, space="PSUM"))

            # qpT[m][:, sc] = (W[g] @ q_b.T) tile:  lhsT=wqT tile, rhs=qT chunk
            for src, w, dst, nm in ((qT, wq, qpT, "q"), (kT, wk, kpT, "k")):
                for sc in range(4):
                    ps = [pps.tile([P, E], f32, tag=f"pj{m}", name=f"pj{nm}{sc}{m}")
                          for m in range(ET)]
                    for dt_ in range(DT):
                        chunk = inp.tile([P, E], mmdt, tag="chunk",
                                         name=f"ch{nm}{sc}{dt_}")
                        nc.sync.dma_start(
                            chunk, src[dt_ * P:(dt_ + 1) * P,
                                       sc * E:(sc + 1) * E])
                        for m in range(ET):
                            nc.tensor.matmul(
                                ps[m], mm(w[:, dt_, m * P:(m + 1) * P]),
                                mm(chunk), start=(dt_ == 0),
                                stop=(dt_ == DT - 1))
                    for m in range(ET):
                        nc.vector.tensor_copy(
                            dst[m][:, sc * E:(sc + 1) * E], ps[m])

            # vp[st] = v_blk @ W[g].T : lhsT = vT chunk [128d,128s], rhs = wk
            for st in range(ST):
                psv = pps.tile([P, E], f32, tag=f"pj{st % ET}", name=f"pv{st}")
                for dt_ in range(DT):
                    vch = inp.tile([P, P], mmdt, tag="vchunk",
                                   name=f"chv{st}{dt_}")
                    nc.sync.dma_start(
                        vch, vT[dt_ * P:(dt_ + 1) * P, st * P:(st + 1) * P])
                    nc.tensor.matmul(psv, mm(vch), mm(wk[:, dt_, :]),
                                     start=(dt_ == 0), stop=(dt_ == DT - 1))
                nc.vector.tensor_copy(vp[st], psv)

        # ------------------------------------------------ attention
        with ExitStack() as actx:
            scp = actx.enter_context(
                tc.tile_pool(name="scp", bufs=2, space="PSUM"))
            otp = actx.enter_context(
                tc.tile_pool(name="otp", bufs=1, space="PSUM"))
            atu = actx.enter_context(tc.tile_pool(name="atu", bufs=3))
            atuTp = actx.enter_context(tc.tile_pool(name="atuTp", bufs=3))
            osb = actx.enter_context(tc.tile_pool(name="osb", bufs=2))
            dn = actx.enter_context(tc.tile_pool(name="dn", bufs=4))

            for h in range(NH):
                m, bp = divmod(h, 2)
                bp *= HD
                qh = qpT[m][bp:bp + HD, :]   # [64, 2048] head-h q-proj (e,s)
                kh = kpT[m][bp:bp + HD, :]

                # ---- Phase A: att rows (softmax over free axis) ----
                for qt in range(ST):
                    lhs_q = qh[:, qt * P:(qt + 1) * P]
                    at = atu.tile([P, S], f32, tag="atu", name=f"at{h}_{qt}")
                    dp = dn.tile([P, 2], f32, tag="dn", name=f"dn{h}_{qt}")
                    for kk in range(2):
                        ps = scp.tile([P, 1024], f32, tag="sc",
                                      name=f"sa{h}_{qt}_{kk}")
                        for cc in range(2):
                            col = kk * 1024 + cc * E
                            nc.tensor.matmul(
                                ps[:, cc * E:(cc + 1) * E], mm(lhs_q),
                                mm(kh[:, col:col + E]), start=True, stop=True)
                        nc.scalar.activation(
                            at[:, kk * 1024:(kk + 1) * 1024], ps, EXP,
                            scale=0.125, accum_out=dp[:, kk:kk + 1])
                    rc = recip[:, h, qt:qt + 1]
                    nc.vector.tensor_add(rc, dp[:, 0:1], dp[:, 1:2])
                    nc.vector.reciprocal(rc, rc)
                    nc.vector.tensor_scalar_mul(at, at, rc)
                    nc.sync.dma_start(att[h, qt * P:(qt + 1) * P, :], at)

                # ---- Phase B: sdpa via transposed scores ----
                ot = [otp.tile([HD, 1024], f32, tag=f"ot{qq}",
                               name=f"ot{h}_{qq}") for qq in range(2)]
                for kt in range(ST):
                    lhs_k = kh[:, kt * P:(kt + 1) * P]
                    aT = atuTp.tile([P, S], mmdt, tag="atuT",
                                    name=f"aT{h}_{kt}")
                    for qq in range(2):
                        ps2 = scp.tile([P, 1024], f32, tag="sc",
                                       name=f"sb{h}_{kt}_{qq}")
                        for cc in range(2):
                            col = qq * 1024 + cc * E
                            nc.tensor.matmul(
                                ps2[:, cc * E:(cc + 1) * E], mm(lhs_k),
                                mm(qh[:, col:col + E]), start=True, stop=True)
                        nc.scalar.activation(
                            aT[:, qq * 1024:(qq + 1) * 1024], ps2, EXP,
                            scale=0.125)
                    vh = vp[kt][:, h * HD:(h + 1) * HD]
                    for qq in range(2):
                        for cc in range(2):
                            col = qq * 1024 + cc * E
                            nc.tensor.matmul(
                                ot[qq][:, cc * E:(cc + 1) * E], mm(vh),
                                mm(aT[:, col:col + E]), start=(kt == 0),
                                stop=(kt == ST - 1))

                # outT [64q-part? no: 64dh, 2048q] -> transpose + normalize
                ob = osb.tile([HD, S], f32, tag="osb", name=f"ob{h}")
                for qq in range(2):
                    nc.vector.tensor_copy(
                        ob[:, qq * 1024:(qq + 1) * 1024], ot[qq])
                for qt in range(ST):
                    tp = scp.tile([P, HD], f32, tag="sc", name=f"tp{h}_{qt}")
                    nc.tensor.transpose(
                        tp, ob[:, qt * P:(qt + 1) * P], ident[0:HD, 0:HD])
                    nc.vector.tensor_scalar_mul(
                        outs[:, qt, h * HD:(h + 1) * HD], tp,
                        recip[:, h, qt:qt + 1])

            nc.sync.dma_start(out.rearrange("(t p) e -> p t e", p=P), outs)

    nc.compile()
    return nc


def _get_nc():
    mm_dt = os.environ.get("KERNEL_MM_DT", "float32r")
    if mm_dt not in _CACHE:
        _CACHE[mm_dt] = _build_nc(mm_dt)
    return _CACHE[mm_dt]


def make_in_maps(q, k, v, WQ, WK):
    q = np.asarray(q, np.float32)
    k = np.asarray(k, np.float32)
    v = np.asarray(v, np.float32)
    WQ = np.asarray(WQ, np.float32)
    WK = np.asarray(WK, np.float32)
    in_maps = []
    for c in range(N_CORES):
        b, g = divmod(c, 2)
        in_maps.append({
            "qT": np.ascontiguousarray(q[b].T),
            "kT": np.ascontiguousarray(k[b].T),
            "vT": np.ascontiguousarray(v[b].T),
            "wqT": np.ascontiguousarray(WQ[g * E:(g + 1) * E].T),
            "wkT": np.ascontiguousarray(WK[g * E:(g + 1) * E].T),
        })
    return in_maps


def assemble(results):
    out = np.empty((4, S, D), np.float32)
    att = np.empty((4, 2 * NH, S, S), np.float32)
    for c in range(N_CORES):
        b, g = divmod(c, 2)
        out[b, :, g * E:(g + 1) * E] = results[c]["out"]
        att[b, g * NH:(g + 1) * NH] = results[c]["att"]
    return out, att


def kernel(q, k, v, WQ, WK, WV=None, batch_size=None, **_unused):
    from concourse.bass_utils import run_bass_kernel_spmd

    nc = _get_nc()
    in_maps = make_in_maps(q, k, v, WQ, WK)
    res = run_bass_kernel_spmd(nc, in_maps, list(range(N_CORES)))
    return assemble(res.results)


# revision 16
# speedup vs baseline: 22.6961x; 22.6961x over previous
"""Trainium2 Bass kernel for nn_MultiHeadAttention_66486093742402.

Reference computation (note source bug faithfully reproduced: v is projected
with WK; WV is unused):
    qp = q @ WQ.T ; kp = k @ WK.T ; vp = v @ WK.T          (per batch b)
    per head h: scores = (qh @ kh.T) / 8 ; att = softmax(scores)
                sdpa = att @ vh
    out[b,s,h*64+d] = sdpa[h,s,d] ;  returns (out, att)

Sharding: 8 cores = 4 batches x 2 head-groups (8 heads each).
Per core inputs (host-prepared, transposed):
    qT,kT,vT [1024,2048] = q[b].T ...,  wqT,wkT [1024,512] = W[g*512:(g+1)*512].T
Per core outputs: att [8,2048,2048], out [2048,512].

On-chip plan (per core):
  projections (4 psum banks, coexist with the attention scores pool):
     qpT/kpT [512e, 2048s] (e on partitions), vp [2048s, 512e]
  per head h (e-tile m=h//2, base partition bp=(h%2)*64):
    Phase A (att output): per q-tile: scores[128q,2048k] = qpT_blk.T @ kpT,
       exp via ScalarE (scale=0.125, accum_out -> denom), recip, normalize
       (VectorE), DMA att row-block (contiguous 1MB).
    Phase B (sdpa): per k-tile: scoresT[128k,2048q] = kpT_blk.T @ qpT, exp ->
       attuT, accumulate outT[64dh,2048q] = vh.T @ attuT over k-tiles in PSUM
       ([64,1024] x2 halves), then PE-transpose 128x64 blocks, normalize by
       recip, DMA out blocks.
Emission order keeps ScalarE (the bottleneck: 2 exp passes over all scores)
busy from ~55us on: q,k projections; A(0..3) with the v-projection tucked in;
then B(h) with A(h+4) interleaved.
All matmuls run as float32r (full-rate fp32; set KERNEL_MM_DT=float32 for the
4x-slower exact-fp32 fallback).
"""

import os
import sys
from contextlib import ExitStack

import numpy as np

sys.path.insert(0, "/opt/trn_rl_repo")

P = 128          # partitions
S = 2048         # sequence
D = 1024         # model dim
E = 512          # per-core output dim (8 heads x 64)
HD = 64          # head depth
NH = 8           # heads per core
ST = S // P      # 16 seq tiles
DT = D // P      # 8 contraction tiles
ET = E // P      # 4 e-tiles
N_CORES = 8

_CACHE = {}


def _build_nc(mm_dt: str):
    import concourse.tile as tile
    from concourse import bacc, mybir
    from concourse.masks import make_identity

    f32 = mybir.dt.float32
    mmdt = getattr(mybir.dt, mm_dt)
    EXP = mybir.ActivationFunctionType.Exp

    def mm(ap):
        return ap

    nc = bacc.Bacc("TRN2", target_bir_lowering=False, debug=False,
                   enable_asserts=False)

    qT = nc.dram_tensor("qT", [D, S], mmdt, kind="ExternalInput").ap()
    kT = nc.dram_tensor("kT", [D, S], mmdt, kind="ExternalInput").ap()
    vT = nc.dram_tensor("vT", [D, S], mmdt, kind="ExternalInput").ap()
    wqT = nc.dram_tensor("wqT", [D, E], mmdt, kind="ExternalInput").ap()
    wkT = nc.dram_tensor("wkT", [D, E], mmdt, kind="ExternalInput").ap()
    att = nc.dram_tensor("att", [NH, S, S], f32, kind="ExternalOutput").ap()
    out = nc.dram_tensor("out", [S, E], f32, kind="ExternalOutput").ap()

    with tile.TileContext(nc) as tc, ExitStack() as ctx:
        persist = ctx.enter_context(tc.tile_pool(name="persist", bufs=1))
        qpT = [persist.tile([P, S], mmdt, tag=f"qpT{m}", name=f"qpT{m}")
               for m in range(ET)]
        kpT = [persist.tile([P, S], mmdt, tag=f"kpT{m}", name=f"kpT{m}")
               for m in range(ET)]
        vp = [persist.tile([P, E], mmdt, tag=f"vp{t}", name=f"vp{t}")
              for t in range(ST)]
        recip = persist.tile([P, NH, ST], f32, tag="recip", name="recip")
        ident = persist.tile([P, P], f32, tag="ident", name="ident")
        make_identity(nc, ident)

        # Attention pools allocated up front (before the projection pools) so
        # attention Phase A can overlap the projection tail: scp (4 psum
        # banks) + pps (4 banks) coexist.
        scp = ctx.enter_context(tc.tile_pool(name="scp", bufs=2, space="PSUM"))
        atu = ctx.enter_context(tc.tile_pool(name="atu", bufs=3))
        atuTp = ctx.enter_context(tc.tile_pool(name="atuTp", bufs=3))
        osb = ctx.enter_context(tc.tile_pool(name="osb", bufs=2))
        dn = ctx.enter_context(tc.tile_pool(name="dn", bufs=4))
        otb = ctx.enter_context(tc.tile_pool(name="otb", bufs=4))

        # ---------------------------------------------------------- emitters
        ot_tiles = {}
        otp_ref = []

        def emit_A(h, qt):
            m, bp = divmod(h, 2)
            bp *= HD
            qh = qpT[m][bp:bp + HD, :]
            kh = kpT[m][bp:bp + HD, :]
            lhs_q = qh[:, qt * P:(qt + 1) * P]
            at = atu.tile([P, S], f32, tag="atu", name=f"at{h}_{qt}")
            dp = dn.tile([P, 2], f32, tag="dn", name=f"dn{h}_{qt}")
            for kk in range(2):
                ps = scp.tile([P, 1024], f32, tag="sc",
                              name=f"sa{h}_{qt}_{kk}")
                for cc in range(2):
                    col = kk * 1024 + cc * E
                    nc.tensor.matmul(
                        ps[:, cc * E:(cc + 1) * E], mm(lhs_q),
                        mm(kh[:, col:col + E]), start=True, stop=True)
                nc.scalar.activation(
                    at[:, kk * 1024:(kk + 1) * 1024], ps, EXP,
                    scale=0.125, accum_out=dp[:, kk:kk + 1])
            rc = recip[:, h, qt:qt + 1]
            nc.vector.tensor_add(rc, dp[:, 0:1], dp[:, 1:2])
            nc.vector.reciprocal(rc, rc)
            nc.vector.tensor_scalar_mul(at, at, rc)
            nc.sync.dma_start(att[h, qt * P:(qt + 1) * P, :], at)

        def emit_B(h, kt):
            otp = otp_ref[0]
            m, bp = divmod(h, 2)
            bp *= HD
            qh = qpT[m][bp:bp + HD, :]
            kh = kpT[m][bp:bp + HD, :]
            if kt == 0:
                ot_tiles[h] = [otp.tile([HD, 1024], f32, tag=f"ot{qq}",
                                        name=f"ot{h}_{qq}")
                               for qq in range(2)]
            ot = ot_tiles[h]
            lhs_k = kh[:, kt * P:(kt + 1) * P]
            aT = atuTp.tile([P, S], mmdt, tag="atuT", name=f"aT{h}_{kt}")
            for qq in range(2):
                ps2 = scp.tile([P, 1024], f32, tag="sc",
                               name=f"sb{h}_{kt}_{qq}")
                for cc in range(2):
                    col = qq * 1024 + cc * E
                    nc.tensor.matmul(
                        ps2[:, cc * E:(cc + 1) * E], mm(lhs_k),
                        mm(qh[:, col:col + E]), start=True, stop=True)
                nc.scalar.activation(
                    aT[:, qq * 1024:(qq + 1) * 1024], ps2, EXP, scale=0.125)
            vh = vp[kt][:, h * HD:(h + 1) * HD]
            for qq in range(2):
                for cc in range(2):
                    col = qq * 1024 + cc * E
                    nc.tensor.matmul(
                        ot[qq][:, cc * E:(cc + 1) * E], mm(vh),
                        mm(aT[:, col:col + E]), start=(kt == 0),
                        stop=(kt == ST - 1))

        def emit_epilogue(h):
            otp = otp_ref[0]
            ot = ot_tiles.pop(h)
            ob = osb.tile([HD, S], f32, tag="osb", name=f"ob{h}")
            for qq in range(2):
                nc.vector.tensor_copy(
                    ob[:, qq * 1024:(qq + 1) * 1024], ot[qq])
            for qt in range(ST):
                tp = otp.tile([P, HD], f32, tag=f"ot{qt % 2}",
                              name=f"tp{h}_{qt}")
                nc.tensor.transpose(
                    tp, ob[:, qt * P:(qt + 1) * P], ident[0:HD, 0:HD])
                ob2 = otb.tile([P, HD], f32, tag="otb", name=f"ob2{h}_{qt}")
                nc.vector.tensor_scalar_mul(ob2, tp, recip[:, h, qt:qt + 1])
                nc.sync.dma_start(
                    out[qt * P:(qt + 1) * P, h * HD:(h + 1) * HD], ob2)

        # ------------------------------------------------ projections + A
        with ExitStack() as pctx:
            wpool = pctx.enter_context(tc.tile_pool(name="wpool", bufs=1))
            wq = wpool.tile([P, DT, E], mmdt, tag="wq", name="wq")
            wk = wpool.tile([P, DT, E], mmdt, tag="wk", name="wk")
            nc.sync.dma_start(wq, wqT.rearrange("(t p) e -> p t e", p=P))
            nc.sync.dma_start(wk, wkT.rearrange("(t p) e -> p t e", p=P))

            inp = pctx.enter_context(tc.tile_pool(name="inp", bufs=6))
            pps = pctx.enter_context(
                tc.tile_pool(name="pps", bufs=1, space="PSUM"))

            # qpT/kpT: 4 column-rounds of 512, 4 psum banks, chunks used once.
            # k first, then q with A(0) units woven in as its columns land.
            def emit_proj(src, w, dst, nm, cr):
                ps = [pps.tile([P, E], f32, tag=f"pj{m}",
                               name=f"pj{nm}{cr}{m}") for m in range(ET)]
                for dt_ in range(DT):
                    chunk = inp.tile([P, E], mmdt, tag="chunk",
                                     name=f"ch{nm}{cr}{dt_}")
                    nc.sync.dma_start(
                        chunk, src[dt_ * P:(dt_ + 1) * P,
                                   cr * E:(cr + 1) * E])
                    for m in range(ET):
                        nc.tensor.matmul(
                            ps[m], mm(w[:, dt_, m * P:(m + 1) * P]),
                            mm(chunk), start=(dt_ == 0),
                            stop=(dt_ == DT - 1))
                for m in range(ET):
                    nc.vector.tensor_copy(
                        dst[m][:, cr * E:(cr + 1) * E], ps[m])

            for cr in range(4):
                emit_proj(kT, wk, kpT, "k", cr)
            for cr in range(4):
                emit_proj(qT, wq, qpT, "q", cr)
                for qt in range(4 * cr, 4 * cr + 4):
                    emit_A(0, qt)
            # A(1) emitted in full before any B: its exps cover the ScalarE
            # while the projection-pool boundary stalls B(0)'s sdpa.
            for qt in range(ST):
                emit_A(1, qt)

            def emit_v():
                # vp[st] = v_blk @ W[g].T : lhsT = vT chunk, rhs = wk
                for r4 in range(4):
                    ps = [pps.tile([P, E], f32, tag=f"pj{st4}",
                                   name=f"pv{r4}{st4}") for st4 in range(4)]
                    for dt_ in range(DT):
                        vch = inp.tile([P, E], mmdt, tag="chunk",
                                       name=f"chv{r4}{dt_}")
                        nc.sync.dma_start(
                            vch, vT[dt_ * P:(dt_ + 1) * P,
                                    r4 * E:(r4 + 1) * E])
                        for st4 in range(4):
                            nc.tensor.matmul(
                                ps[st4], mm(vch[:, st4 * P:(st4 + 1) * P]),
                                mm(wk[:, dt_, :]),
                                start=(dt_ == 0), stop=(dt_ == DT - 1))
                    for st4 in range(4):
                        nc.vector.tensor_copy(vp[r4 * 4 + st4], ps[st4])

            emit_v()

        # ------------------------------------------------ attention B + rest
        # Lead-two interleave: A(h+2) accompanies B(h) so ScalarE always has
        # exp work while B(h+1)'s sdpa waits on the epilogue's psum slots.
        with ExitStack() as actx:
            otp_ref.append(actx.enter_context(
                tc.tile_pool(name="otp", bufs=1, space="PSUM")))
            for h in range(NH):
                for t in range(ST):
                    if h + 2 < NH:
                        emit_A(h + 2, t)
                    emit_B(h, t)
                emit_epilogue(h)

    nc.compile()
    return nc


def _get_nc():
    mm_dt = os.environ.get("KERNEL_MM_DT", "float32r")
    if mm_dt not in _CACHE:
        _CACHE[mm_dt] = _build_nc(mm_dt)
    return _CACHE[mm_dt]


def make_in_maps(q, k, v, WQ, WK):
    q = np.asarray(q, np.float32)
    k = np.asarray(k, np.float32)
    v = np.asarray(v, np.float32)
    WQ = np.asarray(WQ, np.float32)
    WK = np.asarray(WK, np.float32)
    in_maps = []
    for c in range(N_CORES):
        b, g = divmod(c, 2)
        in_maps.append({
            "qT": np.ascontiguousarray(q[b].T),
            "kT": np.ascontiguousarray(k[b].T),
            "vT": np.ascontiguousarray(v[b].T),
            "wqT": np.ascontiguousarray(WQ[g * E:(g + 1) * E].T),
            "wkT": np.ascontiguousarray(WK[g * E:(g + 1) * E].T),
        })
    return in_maps


def assemble(results):
    out = np.empty((4, S, D), np.float32)
    att = np.empty((4, 2 * NH, S, S), np.float32)
    for c in range(N_CORES):
        b, g = divmod(c, 2)
        out[b, :, g * E:(g + 1) * E] = results[c]["out"]
        att[b, g * NH:(g + 1) * NH] = results[c]["att"]
    return out, att


def kernel(q, k, v, WQ, WK, WV=None, batch_size=None, **_unused):
    from concourse.bass_utils import run_bass_kernel_spmd

    nc = _get_nc()
    in_maps = make_in_maps(q, k, v, WQ, WK)
    res = run_bass_kernel_spmd(nc, in_maps, list(range(N_CORES)))
    return assemble(res.results)
